# revision 16
# baseline (speedup 1.0000x reference)
"""AGREE group-recommendation kernel for TRN2 (8 cores, data-parallel over groups).

v5: packed-score dataflow respecting PE tile-position constraints.
Per supertile (128 groups sorted by length, cap L, cw = 16*L):
  mT [E=128, 128*L] bf16 via one transposed dma_gather.
  hid: 8 chunks (16 groups each) at 32-partition offsets across 2 PSUM banks;
    W1a/W1b zero-padded to [E,32] so dead bands are exact zeros.
    relu on ACT costs 2*cw (not 8*cw like replicated layouts).
  s [4, cw] x2 = W2blk4.T @ hid  (block-diag W2 at 32-offsets).
  ex = exp(s) [8, cw]; exm = ex*mask01; den = windowed reduce; dre = 1/den.
  exm_rep [128, cw] per chunk via row-selector matmul (uones) -> PSUM.
  prod = mT * exm_rep (DVE) -> bf16; pooled: L identity-matmuls accumulate
    member columns into [128, 128] PSUM (unnormalized).
  drep [128, 16] per chunk via selector matmul; gv = pooled*drep (normalize).
Batched tail: gv += gT; el = gv*iT; h2 = relu([el,gv,iT]@P1); sigmoid(h2@P2).
"""
import numpy as np
import ml_dtypes

import concourse.bass as bass
import concourse.mybir as mybir
import concourse.tile as tile
from concourse import bacc

F32 = mybir.dt.float32
BF16 = mybir.dt.bfloat16
I32 = mybir.dt.int32
I16 = mybir.dt.int16
AF = mybir.ActivationFunctionType
OP = mybir.AluOpType

B_L = 1024      # groups per core
M = 32          # members per group
E = 128
NST = 8         # supertiles per core
ST_G = 128      # groups per supertile
NCH = 8         # chunks per supertile (16 groups each)
GPC = 16        # groups per chunk
NU = 200000
NI = 50000
NG = 20000
NQ = 2          # SWDGE queues
CAPS = (32, 30, 27, 24, 21, 17, 14, 10)  # fallback; data_caps used at runtime


def lane_split(caps):  # compat stub for test.py print
    return tuple((L, 0) for L in caps)


def build_kernel(num_devices=8, loop_K=0, caps=CAPS):
    nc = bacc.Bacc("TRN2", target_bir_lowering=False, debug=False,
                   num_devices=num_devices, num_swdge_queues=NQ)
    ap = {}
    def dram(name, shape, dt, kind="ExternalInput"):
        ap[name] = nc.dram_tensor(name, shape, dt, kind=kind).ap()
        return ap[name]

    tot_t = sum(caps)                  # 128-idx tiles total
    tot_w = sum(GPC * L for L in caps)  # packed mask cols
    user = dram("user_emb", [NU, E], BF16)
    gtab = dram("group_emb", [NG, E], BF16)
    itab = dram("item_emb", [NI, E], BF16)
    midx16 = dram("midx16", [128, 8 * tot_t], I16)
    gidx16 = dram("gidx16", [128, B_L // 16], I16)
    iidx16 = dram("iidx16", [128, B_L // 16], I16)
    maskb = dram("maskb", [NCH, tot_w], BF16)
    w1a = dram("W1a32", [E, 32], BF16)
    w1b = dram("W1b32", [E, 32], BF16)
    w2blk = dram("W2blk4", [128, 4], BF16)
    msel = dram("msel", [NCH, NCH], BF16)
    b1rep = dram("b1rep32", [128, 1], F32)
    ident = dram("ident", [128, 128], BF16)
    uones = dram("uones", [4, 4 * 128], BF16)
    p1a = dram("P1a", [E, 16], BF16)
    p1b = dram("P1b", [E, 16], BF16)
    p1c = dram("P1c", [E, 16], BF16)
    p1v = dram("p1v", [16, 1], F32)
    p2m = dram("P2", [16, 1], BF16)
    p2v = dram("p2v", [1, 1], F32)
    out = dram("out", [1, B_L], F32, kind="ExternalOutput")

    with tile.TileContext(nc) as tc:
        with (
            tc.tile_pool(name="cst", bufs=1) as cst,
            tc.tile_pool(name="mT", bufs=5) as mTp,
            tc.tile_pool(name="sm", bufs=2) as smp,    # packed [8, cw] tiles
            tc.tile_pool(name="hsb", bufs=2) as hsbp,  # hid sbuf
            tc.tile_pool(name="pr", bufs=2) as prp,    # prod sbuf
            tc.tile_pool(name="gv", bufs=2) as gvp,    # gv/el ring
            tc.tile_pool(name="hps", bufs=2, space="PSUM") as hpsp,   # 2 banks
            tc.tile_pool(name="sps", bufs=1, space="PSUM") as spsp,   # 1 bank
            tc.tile_pool(name="wrp", bufs=2, space="PSUM") as wrpp,   # 4 banks
            tc.tile_pool(name="plp", bufs=1, space="PSUM") as plpp,   # 1 bank
        ):
            # ---- constants ----
            def cload(name, shape, dt):
                t = cst.tile(shape, dt, tag=name)
                nc.sync.dma_start(out=t[:], in_=ap[name][:])
                return t

            midx16_sb = cload("midx16", [128, 8 * tot_t], I16)
            gidx16_sb = cload("gidx16", [128, B_L // 16], I16)
            iidx16_sb = cload("iidx16", [128, B_L // 16], I16)
            w1a_sb = cload("W1a32", [E, 32], BF16)
            w1b_sb = cload("W1b32", [E, 32], BF16)
            w2blk_sb = cload("W2blk4", [128, 4], BF16)
            msel_sb = cload("msel", [NCH, NCH], BF16)
            b1rep_sb = cload("b1rep32", [128, 1], F32)
            ident_sb = cload("ident", [128, 128], BF16)
            uones_sb = cload("uones", [4, 4 * 128], BF16)
            p1a_sb = cload("P1a", [E, 16], BF16)
            p1b_sb = cload("P1b", [E, 16], BF16)
            p1c_sb = cload("P1c", [E, 16], BF16)
            p1_sb = cload("p1v", [16, 1], F32)
            p2m_sb = cload("P2", [16, 1], BF16)
            p2v_sb = cload("p2v", [1, 1], F32)

            # item/group embeddings (gathers emitted inside body)
            gT = cst.tile([128, 1, B_L], BF16, tag="gT")
            iT = cst.tile([128, 1, B_L], BF16, tag="iT")

            def gather_iT():
                nc.gpsimd.dma_gather(
                    out_ap=iT[:, :, :], in_ap=itab[:], idxs_ap=iidx16_sb[:],
                    num_idxs=B_L, num_idxs_reg=B_L, elem_size=E,
                    transpose=True, single_packet=False, queue_num=1 % NQ)

            def gather_gT():
                nc.gpsimd.dma_gather(
                    out_ap=gT[:, :, :], in_ap=gtab[:], idxs_ap=gidx16_sb[:],
                    num_idxs=B_L, num_idxs_reg=B_L, elem_size=E,
                    transpose=True, single_packet=False, queue_num=1 % NQ)

            # per-supertile packed masks (loaded once; iteration-invariant)
            mask_tiles = []
            mb_off = 0
            for s, L in enumerate(caps):
                cw = GPC * L
                mt = cst.tile([NCH, cw], BF16, tag=f"mask{s}")
                nc.sync.dma_start(out=mt[:], in_=maskb[:, mb_off:mb_off + cw])
                mask_tiles.append(mt)
                mb_off += cw


            st = {}   # per-supertile live tiles

            def stage_G(s, split=1):
                L = caps[s]
                tb = sum(caps[:s])
                mT = mTp.tile([128, 1, 4096], BF16, tag="mT", name="mT_t")
                h = (L + split - 1) // split
                o = 0
                q = 0
                while o < L:
                    n = min(h, L - o)
                    nc.gpsimd.dma_gather(
                        out_ap=mT[:, :, 128 * o:128 * (o + n)], in_ap=user[:],
                        idxs_ap=midx16_sb[:, 8 * (tb + o):8 * (tb + o + n)],
                        num_idxs=128 * n, num_idxs_reg=128 * n, elem_size=E,
                        transpose=True, single_packet=False, queue_num=q)
                    o += n
                    q = (q + 1) % NQ
                st[(s, "mT")] = mT

            def stage_H(s, h):
                # PE: hid matmuls for half h (chunks 4h..4h+4); ACT: relu
                L = caps[s]
                cw = GPC * L
                g0 = ST_G * s
                mTv = st[(s, "mT")][:, 0, :]
                hp = hpsp.tile([128, 512], F32, space="PSUM", tag="hps",
                               name="hp_t")
                for q in range(4):
                    k = 4 * h + q
                    nc.tensor.matmul(out=hp[32 * q:32 * q + 32, 0:cw],
                                     lhsT=w1a_sb[:],
                                     rhs=mTv[:, k * cw:(k + 1) * cw],
                                     start=True, stop=False,
                                     tile_position=(0, 32 * q))
                for q in range(4):
                    k = 4 * h + q
                    gk = g0 + GPC * k
                    ip_view = (iT[:, 0, gk:gk + GPC]
                               .unsqueeze(2).to_broadcast([E, GPC, L]))
                    nc.tensor.matmul(out=hp[32 * q:32 * q + 32, 0:cw],
                                     lhsT=w1b_sb[:],
                                     rhs=ip_view, start=False, stop=True,
                                     tile_position=(0, 32 * q))
                hid = hsbp.tile([128, 512], BF16, tag="hid", name="hid_t")
                nc.scalar.activation(out=hid[:, 0:cw], in_=hp[:, 0:cw],
                                     func=AF.Relu, bias=b1rep_sb[:, 0:1])
                st[(s, "hid", h)] = hid

            def stage_S(s, h):
                # PE: mask + block-diag W2 -> packed scores [4, cw]
                L = caps[s]
                cw = GPC * L
                hid = st.pop((s, "hid", h))
                sps = spsp.tile([4, 512], F32, space="PSUM", tag="sps",
                                name="sps_t")
                mt = mask_tiles[s]
                nc.tensor.matmul(out=sps[0:4, 0:cw],
                                 lhsT=msel_sb[:, 4 * h:4 * h + 4],
                                 rhs=mt[:, 0:cw], start=True, stop=False)
                nc.tensor.matmul(out=sps[0:4, 0:cw], lhsT=w2blk_sb[:],
                                 rhs=hid[:, 0:cw], start=False, stop=True)
                st[(s, "sps", h)] = sps

            def stage_E(s, h):
                # ACT: exp into half-h columns of the shared ex tile
                L = caps[s]
                cw = GPC * L
                if (s, "ex") not in st:
                    st[(s, "ex")] = smp.tile([4, 1024], BF16, tag="ex",
                                             name="ex_t")
                ex = st[(s, "ex")]
                sps = st.pop((s, "sps", h))
                nc.scalar.activation(out=ex[0:4, 512 * h:512 * h + cw],
                                     in_=sps[0:4, 0:cw], func=AF.Exp)

            def stage_Bdve(s):
                # DVE: den, 1/den, normalized weights (both halves at once)
                L = caps[s]
                cw = GPC * L
                ex = st.pop((s, "ex"))
                exv = ex[0:4, 0:1024].rearrange(
                    "p (h x) -> p h x", h=2)[:, :, 0:cw].rearrange(
                    "p h (g m) -> p h g m", m=L)
                den = smp.tile([4, 2 * GPC], F32, tag="den")
                nc.vector.tensor_reduce(
                    out=den[:].rearrange("p (h g) -> p h g", h=2),
                    in_=exv,
                    axis=mybir.AxisListType.X, op=OP.add)
                dre = smp.tile([4, 2 * GPC], BF16, tag="dre")
                with nc.allow_low_precision(reason="bf16 1/den, rel 4e-3"):
                    nc.vector.reciprocal(out=dre[:], in_=den[:])
                wgt = smp.tile([4, 1024], BF16, tag="wgt")
                nc.vector.tensor_tensor(
                    out=wgt[0:4, 0:1024].rearrange(
                        "p (h x) -> p h x", h=2)[:, :, 0:cw].rearrange(
                        "p h (g m) -> p h g m", m=L),
                    in0=exv,
                    in1=dre[:].rearrange("p (h g) -> p h g", h=2)
                        .unsqueeze(3).to_broadcast([4, 2, GPC, L]),
                    op=OP.mult)
                st[(s, "wgt")] = wgt

            def stage_C1(s, p):
                # PE: broadcast wgt rows of piece p (2 chunks) into PSUM
                L = caps[s]
                cw = GPC * L
                wgt = st[(s, "wgt")]
                wrep = wrpp.tile([128, 2, 512], F32, space="PSUM",
                                 tag="wrep", name="wrep_t")
                for j in range(2):
                    k = 2 * p + j
                    hh, q = divmod(k, 4)
                    nc.tensor.matmul(out=wrep[:, j, 0:cw],
                                     lhsT=uones_sb[:, 128 * q:128 * (q + 1)],
                                     rhs=wgt[0:4, 512 * hh:512 * hh + cw],
                                     start=True, stop=True)
                st[(s, "wrep", p)] = wrep

            def stage_C2(s, p):
                # DVE: prod piece p = mT * wrep -> one big prod tile
                L = caps[s]
                cw = GPC * L
                if (s, "prod") not in st:
                    st[(s, "prod")] = prp.tile([128, NCH, 512], BF16,
                                               tag="prod", name="prod_t")
                prod = st[(s, "prod")]
                wrep = st.pop((s, "wrep", p))
                mTv = st[(s, "mT")][:, 0, :]
                mpc = mTv[:, 2 * p * cw:(2 * p + 2) * cw].rearrange(
                    "p (two c) -> p two c", two=2)
                nc.vector.tensor_tensor(out=prod[:, 2 * p:2 * p + 2, 0:cw],
                                        in0=mpc,
                                        in1=wrep[:, :, 0:cw],
                                        op=OP.mult)

            def stage_C3(s):
                # PE: pooled = sum over members, L accumulating matmuls
                L = caps[s]
                cw = GPC * L
                prod = st.pop((s, "prod"))
                pooled = plpp.tile([128, 128], F32, space="PSUM", tag="plp",
                                   name="plp_t")
                prv = prod[:, :, 0:cw].rearrange("p c (g m) -> p c g m", m=L)
                for m in range(L):
                    nc.tensor.matmul(out=pooled[:, 0:128],
                                     lhsT=ident_sb[:],
                                     rhs=prv[:, :, :, m],
                                     start=(m == 0), stop=(m == L - 1))
                st[(s, "pooled")] = pooled

            def stage_D(s):
                # ACT: drain pooled -> gv_all slice
                g0 = ST_G * s
                gv_all = st["gv"]
                pooled = st.pop((s, "pooled"))
                nc.scalar.copy(out=gv_all[:, g0:g0 + ST_G],
                               in_=pooled[:, 0:128])

            def tail():
                # gv += gT; el = gv * iT   (batched over all 1024 groups)
                gv_all = st.pop("gv")
                el_all = gvp.tile([128, B_L], BF16, tag="el_all")
                nc.vector.tensor_tensor(out=gv_all[:], in0=gv_all[:],
                                        in1=gT[:, 0, :], op=OP.add)
                nc.vector.tensor_tensor(out=el_all[:], in0=gv_all[:],
                                        in1=iT[:, 0, :], op=OP.mult)
                ot = smp.tile([1, B_L], F32, tag="ot")
                for o in (0, 512):
                    h2 = hpsp.tile([128, 512], F32, space="PSUM", tag="hps",
                                   name="h2_t")
                    nc.tensor.matmul(out=h2[0:16, 0:512], lhsT=p1a_sb[:],
                                     rhs=el_all[:, o:o + 512],
                                     start=True, stop=False)
                    nc.tensor.matmul(out=h2[0:16, 0:512], lhsT=p1b_sb[:],
                                     rhs=gv_all[:, o:o + 512],
                                     start=False, stop=False)
                    nc.tensor.matmul(out=h2[0:16, 0:512], lhsT=p1c_sb[:],
                                     rhs=iT[:, 0, o:o + 512],
                                     start=False, stop=True)
                    h2s = hsbp.tile([128, 512], BF16, tag="hid",
                                    name="h2s_t")
                    nc.scalar.activation(out=h2s[0:16, 0:512],
                                         in_=h2[0:16, 0:512],
                                         func=AF.Relu, bias=p1_sb[:, 0:1])
                    ops = spsp.tile([4, 512], F32, space="PSUM", tag="sps",
                                    name="ops_t")
                    nc.tensor.matmul(out=ops[0:1, 0:512], lhsT=p2m_sb[:],
                                     rhs=h2s[0:16, 0:512],
                                     start=True, stop=True)
                    nc.scalar.activation(out=ot[:, o:o + 512],
                                         in_=ops[0:1, 0:512],
                                         func=AF.Sigmoid, bias=p2v_sb[:, 0:1])
                nc.sync.dma_start(out=out[:], in_=ot[:])

            def body(in_loop=False):
                # software-pipelined emission, descending L; first two
                # gathers split across both queues to cut head latency
                st["gv"] = gvp.tile([128, B_L], BF16, tag="gv_all", name="gv_t")
                ordr = list(range(NST))
                stage_G(ordr[0], split=4)
                if not in_loop:
                    gather_iT()
                stage_G(ordr[1], split=2)
                stage_G(ordr[2])
                stage_H(ordr[0], 0)
                stage_H(ordr[0], 1)
                stage_S(ordr[0], 0)
                stage_E(ordr[0], 0)
                stage_S(ordr[0], 1)
                stage_E(ordr[0], 1)
                for i, s in enumerate(ordr):
                    if i + 3 < NST:
                        stage_G(ordr[i + 3])
                    stage_Bdve(s)
                    if i == 4 and not in_loop:
                        gather_gT()
                    n = ordr[i + 1] if i + 1 < NST else None
                    if n is not None:
                        stage_H(n, 0)
                        stage_H(n, 1)
                    stage_C1(s, 0)
                    stage_C1(s, 1)
                    if i >= 1:
                        stage_C3(ordr[i - 1])
                    stage_C2(s, 0)
                    stage_C2(s, 1)
                    stage_C1(s, 2)
                    stage_C1(s, 3)
                    stage_C2(s, 2)
                    stage_C2(s, 3)
                    if n is not None:
                        stage_S(n, 0)
                        stage_E(n, 0)
                        stage_S(n, 1)
                        stage_E(n, 1)
                    if i >= 1:
                        stage_D(ordr[i - 1])
                stage_C3(ordr[-1])
                stage_D(ordr[-1])
                tail()

            if loop_K:
                gather_iT()
                gather_gT()
                with tc.For_i(0, loop_K, 1):
                    body(in_loop=True)
            else:
                body()
    nc.compile()
    return nc


def data_caps(lengths):
    """Exact per-supertile caps = max over cores of each sorted-rank band."""
    lengths = np.asarray(lengths)
    caps = []
    for s in range(NST):
        mx = 1
        for c in range(8):
            l = np.sort(lengths[c * B_L:(c + 1) * B_L])[::-1] + 1
            mx = max(mx, int(l[ST_G * s:ST_G * (s + 1)].max()))
        caps.append(mx)
    return tuple(caps)


def _wrap16(flat):
    return np.ascontiguousarray(
        np.tile(flat.astype(np.int16).reshape(-1, 16).T, (8, 1)))


def prep_core_inputs(user_emb, item_emb, group_emb, W1, b1, W2,
                     P1, p1, P2, p2, groups_c, items_c, member_idx_c,
                     lengths_c, caps=CAPS):
    """Host-side prep of one core's in_map (b2 dropped: softmax-invariant).

    Groups sorted by length desc; supertile s keeps caps[s] member slots per
    group. ALL member gathers go through int16-indexed dma_gather, so the
    user table is permuted to put every gathered row id below 32768."""
    f32 = np.float32
    bf16 = ml_dtypes.bfloat16
    order = np.argsort(-lengths_c, kind="stable")
    groups_c = np.asarray(groups_c)[order]
    items_c = np.asarray(items_c)[order]
    member_idx_c = np.asarray(member_idx_c)[order]
    lengths_c = np.asarray(lengths_c)[order]
    mask_parts = []
    flat_ids = []
    for s, L in enumerate(caps):
        mi = member_idx_c[ST_G * s:ST_G * (s + 1), :L].astype(np.int64)
        flat_ids.append(mi.reshape(-1))
        le = lengths_c[ST_G * s:ST_G * (s + 1)]
        mg = np.where(np.arange(L)[None, :] <= le[:, None],
                      0.0, -40.0).astype(f32)             # [128, L]
        mask_parts.append(mg.reshape(NCH, GPC * L))
    maskb = np.concatenate(mask_parts, axis=1).astype(bf16)
    all_ids = np.concatenate(flat_ids)
    gl_ids = np.unique(all_ids)
    assert gl_ids.size <= 32768, gl_ids.size
    nu = user_emb.shape[0]
    in_gl = np.zeros(nu, bool)
    in_gl[gl_ids] = True
    perm = np.concatenate([gl_ids, np.nonzero(~in_gl)[0]]).astype(np.int64)
    inv = np.empty(nu, np.int32)
    inv[perm] = np.arange(nu, dtype=np.int32)
    user_perm = np.ascontiguousarray(
        np.asarray(user_emb, dtype=f32)[perm].astype(bf16))
    midx16 = _wrap16(inv[all_ids])
    gidx16 = _wrap16(groups_c.astype(np.int64))   # NG=20000 < 32768
    it_ids = np.unique(items_c)
    ni = item_emb.shape[0]
    in_it = np.zeros(ni, bool)
    in_it[it_ids] = True
    iperm = np.concatenate([it_ids, np.nonzero(~in_it)[0]]).astype(np.int64)
    iinv = np.empty(ni, np.int32)
    iinv[iperm] = np.arange(ni, dtype=np.int32)
    item_perm = np.ascontiguousarray(
        np.asarray(item_emb, dtype=f32)[iperm].astype(bf16))
    iidx16 = _wrap16(iinv[items_c.astype(np.int64)])
    # zero-padded W1 halves: [E, 32] with cols 16:32 = 0
    W1a32 = np.zeros((E, 32), f32)
    W1a32[:, :16] = np.asarray(W1[:E], f32)
    W1b32 = np.zeros((E, 32), f32)
    W1b32[:, :16] = np.asarray(W1[E:], f32)
    # block-diag W2 at 32-offsets: rows 32q+j (j<16) of col q hold W2
    W2blk4 = np.zeros((128, 4), f32)
    for q in range(4):
        W2blk4[32 * q:32 * q + 16, q] = np.asarray(W2, f32).reshape(16)
    msel = np.zeros((NCH, NCH), f32)
    for q in range(4):
        msel[q, q] = 1.0
        msel[4 + q, 4 + q] = 1.0
    b1rep32 = np.zeros((128, 1), f32)
    for q in range(4):
        b1rep32[32 * q:32 * q + 16, 0] = np.asarray(b1, f32).reshape(16)
    # row-selector: uones[q', 128q + p] = 1 iff q' == q
    uones = np.zeros((4, 4 * 128), f32)
    for q in range(4):
        uones[q, 128 * q:128 * (q + 1)] = 1.0
    return {
        "user_emb": user_perm,
        "group_emb": np.ascontiguousarray(np.asarray(group_emb, f32)
                                          .astype(bf16)),
        "item_emb": item_perm,
        "midx16": midx16, "gidx16": gidx16, "iidx16": iidx16,
        "maskb": np.ascontiguousarray(maskb),
        "W1a32": np.ascontiguousarray(W1a32).astype(bf16),
        "W1b32": np.ascontiguousarray(W1b32).astype(bf16),
        "W2blk4": np.ascontiguousarray(W2blk4).astype(bf16),
        "b1rep32": b1rep32,
        "msel": np.ascontiguousarray(msel).astype(bf16),
        "ident": np.eye(128, dtype=f32).astype(bf16),
        "uones": np.ascontiguousarray(uones).astype(bf16),
        "P1a": np.ascontiguousarray(P1[:E]).astype(bf16),
        "P1b": np.ascontiguousarray(P1[E:2 * E]).astype(bf16),
        "P1c": np.ascontiguousarray(P1[2 * E:]).astype(bf16),
        "p1v": np.asarray(p1, dtype=f32).reshape(16, 1),
        "P2": np.ascontiguousarray(np.asarray(P2, f32)).astype(bf16),
        "p2v": np.asarray(p2, dtype=f32).reshape(1, 1),
    }, order


def prep_in_maps(inputs, caps=CAPS):
    maps, orders = [], []
    for c in range(8):
        sl = slice(c * B_L, (c + 1) * B_L)
        m, order = prep_core_inputs(
            np.asarray(inputs["user_emb"]), np.asarray(inputs["item_emb"]),
            np.asarray(inputs["group_emb"]),
            np.asarray(inputs["W1"]), np.asarray(inputs["b1"]),
            np.asarray(inputs["W2"]),
            np.asarray(inputs["P1"]), np.asarray(inputs["p1"]),
            np.asarray(inputs["P2"]), np.asarray(inputs["p2"]),
            np.asarray(inputs["groups"])[sl], np.asarray(inputs["items"])[sl],
            np.asarray(inputs["member_idx"])[sl],
            np.asarray(inputs["lengths"])[sl], caps=caps)
        maps.append(m)
        orders.append(order)
    return maps, orders


def assemble_output(results, orders):
    outs = []
    for c in range(8):
        o = np.empty(B_L, np.float32)
        o[orders[c]] = results[c]["out"].reshape(B_L)
        outs.append(o)
    return np.concatenate(outs).reshape(-1, 1)


# ---------------------------------------------------------------------------
# Self-contained entrypoint: kernel(**inputs) -> np.ndarray [8192, 1]
# ---------------------------------------------------------------------------
_NC_CACHE = {}


def _get_nc(caps):
    if caps not in _NC_CACHE:
        _NC_CACHE[caps] = build_kernel(num_devices=8, caps=caps)
    return _NC_CACHE[caps]


def kernel(**inputs) -> np.ndarray:
    from concourse.bass_utils import run_bass_kernel_spmd

    caps = data_caps(np.asarray(inputs["lengths"]))
    nc = _get_nc(caps)
    in_maps, orders = prep_in_maps(inputs, caps=caps)
    res = run_bass_kernel_spmd(nc, in_maps, core_ids=list(range(8)))
    return assemble_output(res.results, orders).astype(np.float32)


# revision 37
# speedup vs baseline: 1.6435x; 1.6435x over previous
"""AGREE group-recommendation kernel for TRN2 (8 cores, data-parallel over groups).

v6: packed-score dataflow + software-pipelined supertile emission.
Groups are sorted by length (desc) per core; supertile s holds 128 groups
capped at caps[s] member slots. Per supertile (nch chunks of gpc groups,
cw = gpc*L <= 512, nch*gpc = 128):
  mT [E=128, 128*L] bf16 via transposed dma_gather (first tiles split
    across SWDGE queues to cut head latency; region-level deps let hid
    matmuls start on partially-arrived data).
  hid: chunks at 32-partition offsets (PE tile_position), W1a/W1b zero-
    padded to [E,32] so dead bands are exact zeros; one relu per bank.
  scores [nbk, cw] = msel@mask(-40) + W2blk@hid  (mask folded into PSUM);
  ex = exp(s) [<=4, 2*512]; den = windowed reduce; dre = 1/den (bf16).
  ex_rep [128, cw] per chunk via row-selector matmul (uones) -> PSUM;
  prod = mT * ex_rep (DVE); pooled: L identity-matmul accumulations (PE);
  drep via selector matmuls; gv = pooled*drep (DVE) + gT (Pool);
  el = gv*iT (Pool). Tail pieces (h2 = relu([el,gv,iT]@P1), out =
  1/(1+exp(-z-p2)) via exp to avoid an ACT table swap) are emitted as
  soon as their supertiles drain.
Stages are emitted software-pipelined across supertiles so no engine
queues behind a cross-engine dependency; gathers run 3 supertiles ahead.
"""
import numpy as np
import ml_dtypes

import concourse.bass as bass
import concourse.mybir as mybir
import concourse.tile as tile
from concourse import bacc

F32 = mybir.dt.float32
BF16 = mybir.dt.bfloat16
I32 = mybir.dt.int32
I16 = mybir.dt.int16
AF = mybir.ActivationFunctionType
OP = mybir.AluOpType

B_L = 1024      # groups per core
M = 32          # members per group
E = 128
NST = 8         # supertiles per core
ST_G = 128      # groups per supertile
NCH = 8         # chunks per supertile (16 groups each)
GPC = 16        # groups per chunk
NU = 200000
NI = 50000
NG = 20000
NQ = 2          # SWDGE queues
CAPS = (32, 30, 27, 24, 21, 17, 14, 10)  # fallback; data_caps used at runtime


def lane_split(caps):  # compat stub for test.py print
    return tuple((L, 0) for L in caps)


def st_cfg(L):
    """(nch, gpc): chunk count and groups/chunk; gpc*L <= 512, nch*gpc=128."""
    if L > 16:
        return 8, 16
    if L > 8:
        return 4, 32
    if L > 4:
        return 2, 64
    return 1, 128


def build_kernel(num_devices=8, loop_K=0, caps=CAPS, reps=1):
    nc = bacc.Bacc("TRN2", target_bir_lowering=False, debug=False,
                   num_devices=num_devices, num_swdge_queues=NQ)
    ap = {}
    def dram(name, shape, dt, kind="ExternalInput"):
        ap[name] = nc.dram_tensor(name, shape, dt, kind=kind).ap()
        return ap[name]

    tot_t = sum(caps)                  # 128-idx tiles total
    tot_w = sum(st_cfg(L)[1] * L for L in caps)  # packed mask cols
    user = dram("user_emb", [NU, E], BF16)
    gtab = dram("group_emb", [NG, E], BF16)
    itab = dram("item_emb", [NI, E], BF16)
    midx16 = dram("midx16", [128, 8 * tot_t], I16)
    gidx16 = dram("gidx16", [128, B_L // 16], I16)
    iidx16 = dram("iidx16", [128, B_L // 16], I16)
    maskb = dram("maskb", [NCH, tot_w], BF16)
    w1a = dram("W1a32", [E, 32], BF16)
    w1b = dram("W1b32", [E, 32], BF16)
    w2blk = dram("W2blk4", [128, 4], BF16)
    msel = dram("msel", [NCH, NCH], BF16)
    b1rep = dram("b1rep32", [128, 1], F32)
    ident = dram("ident", [128, 128], BF16)
    uones = dram("uones", [4, 4 * 128], BF16)
    p1a = dram("P1a", [E, 16], BF16)
    p1b = dram("P1b", [E, 16], BF16)
    p1c = dram("P1c", [E, 16], BF16)
    p1v = dram("p1v", [16, 1], F32)
    p2m = dram("P2", [16, 1], BF16)
    p2v = dram("p2v", [1, 1], F32)
    np2v = dram("np2v", [1, 1], F32)
    out = dram("out", [1, B_L], F32, kind="ExternalOutput")

    with tile.TileContext(nc) as tc:
        with (
            tc.tile_pool(name="cst", bufs=1) as cst,
            tc.tile_pool(name="mT", bufs=5) as mTp,
            tc.tile_pool(name="sm", bufs=2) as smp,    # packed [8, cw] tiles
            tc.tile_pool(name="hsb", bufs=2) as hsbp,  # hid sbuf
            tc.tile_pool(name="pr", bufs=2) as prp,    # prod sbuf
            tc.tile_pool(name="gv", bufs=2) as gvp,    # gv/el ring
            tc.tile_pool(name="hps", bufs=2, space="PSUM") as hpsp,   # 2 banks
            tc.tile_pool(name="sps", bufs=1, space="PSUM") as spsp,   # 1 bank
            tc.tile_pool(name="wrp", bufs=2, space="PSUM") as wrpp,   # 4 banks
            tc.tile_pool(name="plp", bufs=1, space="PSUM") as plpp,   # 1 bank
        ):
            # ---- constants ----
            def cload(name, shape, dt):
                t = cst.tile(shape, dt, tag=name)
                nc.sync.dma_start(out=t[:], in_=ap[name][:])
                return t

            midx16_sb = cst.tile([128, 8 * tot_t], I16, tag="midx16")
            c0 = 8 * caps[0]
            nc.sync.dma_start(out=midx16_sb[:, 0:c0],
                              in_=ap["midx16"][:, 0:c0])
            nc.sync.dma_start(out=midx16_sb[:, c0:],
                              in_=ap["midx16"][:, c0:])
            gidx16_sb = cload("gidx16", [128, B_L // 16], I16)
            iidx16_sb = cload("iidx16", [128, B_L // 16], I16)
            w1a_sb = cload("W1a32", [E, 32], BF16)
            w1b_sb = cload("W1b32", [E, 32], BF16)
            w2blk_sb = cload("W2blk4", [128, 4], BF16)
            msel_sb = cload("msel", [NCH, NCH], BF16)
            b1rep_sb = cload("b1rep32", [128, 1], F32)
            ident_sb = cload("ident", [128, 128], BF16)
            ident_sb = cload("ident", [128, 128], BF16)
            uones_sb = cload("uones", [4, 4 * 128], BF16)
            p1a_sb = cload("P1a", [E, 16], BF16)
            p1b_sb = cload("P1b", [E, 16], BF16)
            p1c_sb = cload("P1c", [E, 16], BF16)
            p1_sb = cload("p1v", [16, 1], F32)
            p2m_sb = cload("P2", [16, 1], BF16)
            p2v_sb = cload("p2v", [1, 1], F32)
            np2v_sb = cload("np2v", [1, 1], F32)

            # item/group embeddings (gathers emitted inside body)
            gT = cst.tile([128, 1, B_L], BF16, tag="gT")
            iT = cst.tile([128, 1, B_L], BF16, tag="iT")

            def gather_iT():
                nc.gpsimd.dma_gather(
                    out_ap=iT[:, :, :], in_ap=itab[:], idxs_ap=iidx16_sb[:],
                    num_idxs=B_L, num_idxs_reg=B_L, elem_size=E,
                    transpose=True, single_packet=False, queue_num=1 % NQ)

            def gather_gT():
                nc.gpsimd.dma_gather(
                    out_ap=gT[:, :, :], in_ap=gtab[:], idxs_ap=gidx16_sb[:],
                    num_idxs=B_L, num_idxs_reg=B_L, elem_size=E,
                    transpose=True, single_packet=False, queue_num=1 % NQ)

            # per-supertile packed masks (loaded once; iteration-invariant)
            mask_tiles = []
            mb_off = 0
            for s, L in enumerate(caps):
                nch_, gpc_ = st_cfg(L)
                cw = gpc_ * L
                mt = cst.tile([NCH, cw], BF16, tag=f"mask{s}")
                nc.sync.dma_start(out=mt[0:nch_, :],
                                  in_=maskb[0:nch_, mb_off:mb_off + cw])
                mask_tiles.append(mt)
                mb_off += cw


            st = {}   # per-supertile live tiles

            def stage_G(s, split=1, tail_from=None, pre=()):
                L = caps[s]
                tb = sum(caps[:s])
                mT = mTp.tile([128, 1, 4096], BF16, tag="mT", name="mT_t")
                h = (L + split - 1) // split
                o = 0
                q = 0
                i_ = 0
                while o < L:
                    if tail_from is not None and i_ == tail_from:
                        for fn in pre:
                            fn()
                    n = min(h, L - o)
                    nc.gpsimd.dma_gather(
                        out_ap=mT[:, :, 128 * o:128 * (o + n)], in_ap=user[:],
                        idxs_ap=midx16_sb[:, 8 * (tb + o):8 * (tb + o + n)],
                        num_idxs=128 * n, num_idxs_reg=128 * n, elem_size=E,
                        transpose=True, single_packet=False, queue_num=q)
                    o += n
                    q = (q + 1) % NQ
                    i_ += 1
                st[(s, "mT")] = mT

            def stage_H(s, h):
                # PE: hid matmuls for bank h; ACT: relu
                L = caps[s]
                nch, gpc = st_cfg(L)
                cw = gpc * L
                nbk = min(4, nch)           # chunks in this bank
                g0 = ST_G * s
                mTv = st[(s, "mT")][:, 0, :]
                hp = hpsp.tile([128, 512], F32, space="PSUM", tag="hps",
                               name="hp_t")
                for q in range(nbk):
                    k = 4 * h + q
                    nc.tensor.matmul(out=hp[32 * q:32 * q + 32, 0:cw],
                                     lhsT=w1a_sb[:],
                                     rhs=mTv[:, k * cw:(k + 1) * cw],
                                     start=True, stop=False,
                                     tile_position=(0, 32 * q))
                for q in range(nbk):
                    k = 4 * h + q
                    gk = g0 + gpc * k
                    ip_view = (iT[:, 0, gk:gk + gpc]
                               .unsqueeze(2).to_broadcast([E, gpc, L]))
                    nc.tensor.matmul(out=hp[32 * q:32 * q + 32, 0:cw],
                                     lhsT=w1b_sb[:],
                                     rhs=ip_view, start=False, stop=True,
                                     tile_position=(0, 32 * q))
                hid = hsbp.tile([128, 512], BF16, tag="hid", name="hid_t")
                nc.scalar.activation(out=hid[0:32 * nbk, 0:cw],
                                     in_=hp[0:32 * nbk, 0:cw],
                                     func=AF.Relu,
                                     bias=b1rep_sb[0:32 * nbk, 0:1])
                st[(s, "hid", h)] = hid

            def stage_S(s, h):
                # PE: mask + block-diag W2 -> packed scores [nbk, cw]
                L = caps[s]
                nch, gpc = st_cfg(L)
                cw = gpc * L
                nbk = min(4, nch)
                hid = st.pop((s, "hid", h))
                sps = spsp.tile([4, 512], F32, space="PSUM", tag="sps",
                                name="sps_t")
                mt = mask_tiles[s]
                nc.tensor.matmul(out=sps[0:nbk, 0:cw],
                                 lhsT=msel_sb[0:nch, 4 * h:4 * h + nbk],
                                 rhs=mt[0:nch, 0:cw], start=True, stop=False)
                nc.tensor.matmul(out=sps[0:nbk, 0:cw],
                                 lhsT=w2blk_sb[0:32 * nbk, 0:nbk],
                                 rhs=hid[0:32 * nbk, 0:cw],
                                 start=False, stop=True)
                st[(s, "sps", h)] = sps

            def stage_E(s, h):
                # ACT: exp into bank-h columns of the shared ex tile
                L = caps[s]
                nch, gpc = st_cfg(L)
                cw = gpc * L
                nbk = min(4, nch)
                if (s, "ex") not in st:
                    st[(s, "ex")] = smp.tile([4, 1024], BF16, tag="ex",
                                             name="ex_t")
                ex = st[(s, "ex")]
                sps = st.pop((s, "sps", h))
                nc.scalar.activation(out=ex[0:nbk, 512 * h:512 * h + cw],
                                     in_=sps[0:nbk, 0:cw], func=AF.Exp)

            def stage_Bdve(s):
                # DVE: den + 1/den (normalization applied at drain)
                L = caps[s]
                nch, gpc = st_cfg(L)
                cw = gpc * L
                nbk = min(4, nch)
                nh = (nch + 3) // 4
                ex = st[(s, "ex")]
                exv = ex[0:nbk, 0:1024].rearrange(
                    "p (h x) -> p h x", h=2)[:, 0:nh, 0:cw].rearrange(
                    "p h (g m) -> p h g m", m=L)
                den = smp.tile([4, 128], F32, tag="den")
                dnv = den[0:nbk, 0:nh * gpc].rearrange(
                    "p (h g) -> p h g", h=nh)
                nc.vector.tensor_reduce(
                    out=dnv, in_=exv,
                    axis=mybir.AxisListType.X, op=OP.add)
                dre = smp.tile([4, 128], BF16, tag="dre")
                with nc.allow_low_precision(reason="bf16 1/den, rel 4e-3"):
                    nc.vector.reciprocal(out=dre[0:nbk, 0:nh * gpc],
                                         in_=den[0:nbk, 0:nh * gpc])
                st[(s, "dre")] = dre

            def stage_C1(s, p):
                # PE: broadcast ex rows of piece p (<=2 chunks) into PSUM
                L = caps[s]
                nch, gpc = st_cfg(L)
                cw = gpc * L
                nbk = min(4, nch)
                npc = max(1, nch // 2)
                ex = st[(s, "ex")] if p < npc - 1 else st.pop((s, "ex"))
                wrep = wrpp.tile([128, 2, 512], F32, space="PSUM",
                                 tag="wrep", name="wrep_t")
                for j in range(min(2, nch)):
                    k = min(2, nch) * p + j
                    hh, q = divmod(k, 4)
                    nc.tensor.matmul(out=wrep[:, j, 0:cw],
                                     lhsT=uones_sb[0:nbk,
                                                   128 * q:128 * (q + 1)],
                                     rhs=ex[0:nbk, 512 * hh:512 * hh + cw],
                                     start=True, stop=True)
                st[(s, "wrep", p)] = wrep

            def stage_C2(s, p):
                # DVE: prod piece p = mT * wrep -> one big prod tile
                L = caps[s]
                nch, gpc = st_cfg(L)
                cw = gpc * L
                nj = min(2, nch)
                if (s, "prod") not in st:
                    st[(s, "prod")] = prp.tile([128, NCH, 512], BF16,
                                               tag="prod", name="prod_t")
                prod = st[(s, "prod")]
                wrep = st.pop((s, "wrep", p))
                mTv = st[(s, "mT")][:, 0, :]
                mpc = mTv[:, nj * p * cw:(nj * p + nj) * cw].rearrange(
                    "p (two c) -> p two c", two=nj)
                nc.vector.tensor_tensor(out=prod[:, nj * p:nj * p + nj, 0:cw],
                                        in0=mpc,
                                        in1=wrep[:, 0:nj, 0:cw],
                                        op=OP.mult)

            def stage_C3(s):
                # PE: pooled = sum over members, L accumulating matmuls
                L = caps[s]
                nch, gpc = st_cfg(L)
                cw = gpc * L
                nbk = min(4, nch)
                prod = st.pop((s, "prod"))
                dre = st.pop((s, "dre"))
                pooled = plpp.tile([128, 256], F32, space="PSUM", tag="plp",
                                   name="plp_t")
                for k in range(nch):
                    hh, q = divmod(k, 4)
                    nc.tensor.matmul(
                        out=pooled[:, 128 + gpc * k:128 + gpc * (k + 1)],
                        lhsT=uones_sb[0:nbk, 128 * q:128 * (q + 1)],
                        rhs=dre[0:nbk, gpc * hh:gpc * (hh + 1)],
                        start=True, stop=True)
                prv = prod[:, 0:nch, 0:cw].rearrange(
                    "p c (g m) -> p c g m", m=L)
                for m in range(L):
                    nc.tensor.matmul(out=pooled[:, 0:128],
                                     lhsT=ident_sb[:],
                                     rhs=prv[:, :, :, m],
                                     start=(m == 0), stop=(m == L - 1))
                st[(s, "pooled")] = pooled

            def stage_D(s):
                # DVE: gv = pooled*(1/den) -> SBUF; Pool: +gT, then el = gv*iT
                g0 = ST_G * s
                gv_all = st["gv"]
                el_all = st["el"]
                pooled = st.pop((s, "pooled"))
                drc = smp.tile([128, 128], BF16, tag="drc")
                nc.scalar.copy(out=drc[:], in_=pooled[:, 128:256])
                nc.vector.tensor_tensor(out=gv_all[:, g0:g0 + ST_G],
                                        in0=pooled[:, 0:128],
                                        in1=drc[:],
                                        op=OP.mult)
                nc.gpsimd.tensor_tensor(out=gv_all[:, g0:g0 + ST_G],
                                        in0=gv_all[:, g0:g0 + ST_G],
                                        in1=gT[:, 0, g0:g0 + ST_G],
                                        op=OP.add)
                nc.gpsimd.tensor_tensor(out=el_all[:, g0:g0 + ST_G],
                                        in0=gv_all[:, g0:g0 + ST_G],
                                        in1=iT[:, 0, g0:g0 + ST_G],
                                        op=OP.mult)

            def tail_half(hh, o=None, w=512):
                # h2 = relu([el,gv,iT]@P1); out = sigmoid(h2@P2), one piece
                o = 512 * hh if o is None else o
                gv_all = st["gv"]
                el_all = st["el"]
                h2 = hpsp.tile([128, 512], F32, space="PSUM", tag="hps",
                               name="h2_t")
                nc.tensor.matmul(out=h2[0:16, 0:w], lhsT=p1a_sb[:],
                                 rhs=el_all[:, o:o + w],
                                 start=True, stop=False)
                nc.tensor.matmul(out=h2[0:16, 0:w], lhsT=p1b_sb[:],
                                 rhs=gv_all[:, o:o + w],
                                 start=False, stop=False)
                nc.tensor.matmul(out=h2[0:16, 0:w], lhsT=p1c_sb[:],
                                 rhs=iT[:, 0, o:o + w],
                                 start=False, stop=True)
                h2s = hsbp.tile([128, 512], BF16, tag="hid",
                                name="h2s_t")
                nc.scalar.activation(out=h2s[0:16, 0:w],
                                     in_=h2[0:16, 0:w],
                                     func=AF.Relu, bias=p1_sb[:, 0:1])
                ops = spsp.tile([4, 512], F32, space="PSUM", tag="sps",
                                name="ops_t")
                nc.tensor.matmul(out=ops[0:1, 0:w], lhsT=p2m_sb[:],
                                 rhs=h2s[0:16, 0:w],
                                 start=True, stop=True)
                # sigmoid via exp (keeps the relu/exp ACT table set):
                # out = 1 / (1 + exp(-(z + p2)))
                ez = smp.tile([1, 512], F32, tag="ez")
                nc.scalar.activation(out=ez[:, 0:w],
                                     in_=ops[0:1, 0:w],
                                     func=AF.Exp, bias=np2v_sb[:, 0:1],
                                     scale=-1.0)
                nc.vector.tensor_scalar_add(out=ez[:, 0:w], in0=ez[:, 0:w],
                                            scalar1=1.0)
                ot = smp.tile([1, 512], F32, tag="ot")
                nc.vector.reciprocal(out=ot[:, 0:w], in_=ez[:, 0:w])
                nc.sync.dma_start(out=out[0:1, o:o + w], in_=ot[:, 0:w])

            def tail_end():
                st.pop("gv")
                st.pop("el")

            def body(in_loop=False):
                # software-pipelined emission, descending L; first two
                # gathers split across both queues to cut head latency
                st["gv"] = gvp.tile([128, B_L], BF16, tag="gv_all", name="gv_t")
                st["el"] = gvp.tile([128, B_L], BF16, tag="el_all", name="el_t")
                wrm = smp.tile([1, 2], F32, tag="wrm")
                nc.vector.memset(wrm[:], 0.0)
                nc.scalar.activation(out=wrm[0:1, 0:1], in_=wrm[0:1, 1:2],
                                     func=AF.Relu)
                nc.scalar.activation(out=wrm[0:1, 0:1], in_=wrm[0:1, 1:2],
                                     func=AF.Exp)
                ordr = list(range(NST))
                stage_G(ordr[0], split=4)
                # PE p-state warmup: dummy matmuls while gathers land
                wps = hpsp.tile([128, 512], F32, space="PSUM", tag="hps",
                                name="wps_t")
                for _ in range(48):
                    nc.tensor.matmul(out=wps[0:32, 0:128], lhsT=w1a_sb[:],
                                     rhs=ident_sb[:, 0:128],
                                     start=True, stop=True)
                if not in_loop:
                    gather_iT()
                stage_G(ordr[1], split=2)
                stage_G(ordr[2])
                def halves(s_):
                    return (st_cfg(caps[s_])[0] + 3) // 4

                def pieces(s_):
                    return max(1, st_cfg(caps[s_])[0] // 2)

                for h in range(halves(ordr[0])):
                    stage_H(ordr[0], h)
                for h in range(halves(ordr[0])):
                    stage_S(ordr[0], h)
                    stage_E(ordr[0], h)
                for i, s in enumerate(ordr):
                    if i + 3 < NST:
                        stage_G(ordr[i + 3])
                    stage_Bdve(s)
                    if i == 0 and not in_loop:
                        gather_gT()
                    n = ordr[i + 1] if i + 1 < NST else None
                    p0 = 0
                    if i == 0:
                        stage_C1(s, 0)
                        stage_C2(s, 0)
                        p0 = 1
                    if n is not None:
                        for h in range(halves(n)):
                            stage_H(n, h)
                    npc = pieces(s)
                    for p in range(p0, npc):
                        stage_C1(s, p)
                        if p == p0 and i >= 1:
                            stage_C3(ordr[i - 1])
                        stage_C2(s, p)
                    if n is not None:
                        for h in range(halves(n)):
                            stage_S(n, h)
                            stage_E(n, h)
                    if i >= 1:
                        stage_D(ordr[i - 1])
                    if i == 5:
                        tail_half(0)
                    if i == 7:
                        tail_half(1, o=512, w=256)
                        tail_half(1, o=768, w=128)
                stage_C3(ordr[-1])
                stage_D(ordr[-1])
                tail_half(1, o=896, w=128)
                tail_end()

            if loop_K:
                gather_iT()
                gather_gT()
                with tc.For_i(0, loop_K, 1):
                    body(in_loop=True)
            else:
                for _ in range(reps):
                    body()
    nc.compile()
    return nc


def data_caps(lengths):
    """Exact per-supertile caps = max over cores of each sorted-rank band."""
    lengths = np.asarray(lengths)
    caps = []
    for s in range(NST):
        mx = 1
        for c in range(8):
            l = np.sort(lengths[c * B_L:(c + 1) * B_L])[::-1] + 1
            mx = max(mx, int(l[ST_G * s:ST_G * (s + 1)].max()))
        caps.append(mx)
    return tuple(caps)


def _wrap16(flat):
    return np.ascontiguousarray(
        np.tile(flat.astype(np.int16).reshape(-1, 16).T, (8, 1)))


def prep_core_inputs(user_emb, item_emb, group_emb, W1, b1, W2,
                     P1, p1, P2, p2, groups_c, items_c, member_idx_c,
                     lengths_c, caps=CAPS):
    """Host-side prep of one core's in_map (b2 dropped: softmax-invariant).

    Groups sorted by length desc; supertile s keeps caps[s] member slots per
    group. ALL member gathers go through int16-indexed dma_gather, so the
    user table is permuted to put every gathered row id below 32768."""
    f32 = np.float32
    bf16 = ml_dtypes.bfloat16
    order = np.argsort(-lengths_c, kind="stable")
    groups_c = np.asarray(groups_c)[order]
    items_c = np.asarray(items_c)[order]
    member_idx_c = np.asarray(member_idx_c)[order]
    lengths_c = np.asarray(lengths_c)[order]
    mask_parts = []
    flat_ids = []
    for s, L in enumerate(caps):
        mi = member_idx_c[ST_G * s:ST_G * (s + 1), :L].astype(np.int64)
        flat_ids.append(mi.reshape(-1))
        le = lengths_c[ST_G * s:ST_G * (s + 1)]
        mg = np.where(np.arange(L)[None, :] <= le[:, None],
                      0.0, -40.0).astype(f32)             # [128, L]
        nch, gpc = st_cfg(L)
        mp = np.zeros((NCH, gpc * L), f32)
        mp[:nch] = mg.reshape(nch, gpc * L)
        mask_parts.append(mp)
    maskb = np.concatenate(mask_parts, axis=1).astype(bf16)
    all_ids = np.concatenate(flat_ids)
    gl_ids = np.unique(all_ids)
    assert gl_ids.size <= 32768, gl_ids.size
    nu = user_emb.shape[0]
    in_gl = np.zeros(nu, bool)
    in_gl[gl_ids] = True
    perm = np.concatenate([gl_ids, np.nonzero(~in_gl)[0]]).astype(np.int64)
    inv = np.empty(nu, np.int32)
    inv[perm] = np.arange(nu, dtype=np.int32)
    user_perm = np.ascontiguousarray(
        np.asarray(user_emb, dtype=f32)[perm].astype(bf16))
    midx16 = _wrap16(inv[all_ids])
    gidx16 = _wrap16(groups_c.astype(np.int64))   # NG=20000 < 32768
    it_ids = np.unique(items_c)
    ni = item_emb.shape[0]
    in_it = np.zeros(ni, bool)
    in_it[it_ids] = True
    iperm = np.concatenate([it_ids, np.nonzero(~in_it)[0]]).astype(np.int64)
    iinv = np.empty(ni, np.int32)
    iinv[iperm] = np.arange(ni, dtype=np.int32)
    item_perm = np.ascontiguousarray(
        np.asarray(item_emb, dtype=f32)[iperm].astype(bf16))
    iidx16 = _wrap16(iinv[items_c.astype(np.int64)])
    # zero-padded W1 halves: [E, 32] with cols 16:32 = 0
    W1a32 = np.zeros((E, 32), f32)
    W1a32[:, :16] = np.asarray(W1[:E], f32)
    W1b32 = np.zeros((E, 32), f32)
    W1b32[:, :16] = np.asarray(W1[E:], f32)
    # block-diag W2 at 32-offsets: rows 32q+j (j<16) of col q hold W2
    W2blk4 = np.zeros((128, 4), f32)
    for q in range(4):
        W2blk4[32 * q:32 * q + 16, q] = np.asarray(W2, f32).reshape(16)
    msel = np.zeros((NCH, NCH), f32)
    for q in range(4):
        msel[q, q] = 1.0
        msel[4 + q, 4 + q] = 1.0
    b1rep32 = np.zeros((128, 1), f32)
    for q in range(4):
        b1rep32[32 * q:32 * q + 16, 0] = np.asarray(b1, f32).reshape(16)
    # row-selector: uones[q', 128q + p] = 1 iff q' == q
    uones = np.zeros((4, 4 * 128), f32)
    for q in range(4):
        uones[q, 128 * q:128 * (q + 1)] = 1.0
    return {
        "user_emb": user_perm,
        "group_emb": np.ascontiguousarray(np.asarray(group_emb, f32)
                                          .astype(bf16)),
        "item_emb": item_perm,
        "midx16": midx16, "gidx16": gidx16, "iidx16": iidx16,
        "maskb": np.ascontiguousarray(maskb),
        "W1a32": np.ascontiguousarray(W1a32).astype(bf16),
        "W1b32": np.ascontiguousarray(W1b32).astype(bf16),
        "W2blk4": np.ascontiguousarray(W2blk4).astype(bf16),
        "b1rep32": b1rep32,
        "msel": np.ascontiguousarray(msel).astype(bf16),
        "ident": np.eye(128, dtype=f32).astype(bf16),
        "uones": np.ascontiguousarray(uones).astype(bf16),
        "P1a": np.ascontiguousarray(P1[:E]).astype(bf16),
        "P1b": np.ascontiguousarray(P1[E:2 * E]).astype(bf16),
        "P1c": np.ascontiguousarray(P1[2 * E:]).astype(bf16),
        "p1v": np.asarray(p1, dtype=f32).reshape(16, 1),
        "P2": np.ascontiguousarray(np.asarray(P2, f32)).astype(bf16),
        "p2v": np.asarray(p2, dtype=f32).reshape(1, 1),
        "np2v": -np.asarray(p2, dtype=f32).reshape(1, 1),
    }, order


def prep_in_maps(inputs, caps=CAPS):
    maps, orders = [], []
    for c in range(8):
        sl = slice(c * B_L, (c + 1) * B_L)
        m, order = prep_core_inputs(
            np.asarray(inputs["user_emb"]), np.asarray(inputs["item_emb"]),
            np.asarray(inputs["group_emb"]),
            np.asarray(inputs["W1"]), np.asarray(inputs["b1"]),
            np.asarray(inputs["W2"]),
            np.asarray(inputs["P1"]), np.asarray(inputs["p1"]),
            np.asarray(inputs["P2"]), np.asarray(inputs["p2"]),
            np.asarray(inputs["groups"])[sl], np.asarray(inputs["items"])[sl],
            np.asarray(inputs["member_idx"])[sl],
            np.asarray(inputs["lengths"])[sl], caps=caps)
        maps.append(m)
        orders.append(order)
    return maps, orders


def assemble_output(results, orders):
    outs = []
    for c in range(8):
        o = np.empty(B_L, np.float32)
        o[orders[c]] = results[c]["out"].reshape(B_L)
        outs.append(o)
    return np.concatenate(outs).reshape(-1, 1)


# ---------------------------------------------------------------------------
# Self-contained entrypoint: kernel(**inputs) -> np.ndarray [8192, 1]
# ---------------------------------------------------------------------------
_NC_CACHE = {}


def _get_nc(caps):
    if caps not in _NC_CACHE:
        _NC_CACHE[caps] = build_kernel(num_devices=8, caps=caps)
    return _NC_CACHE[caps]


def kernel(**inputs) -> np.ndarray:
    from concourse.bass_utils import run_bass_kernel_spmd

    caps = data_caps(np.asarray(inputs["lengths"]))
    nc = _get_nc(caps)
    in_maps, orders = prep_in_maps(inputs, caps=caps)
    res = run_bass_kernel_spmd(nc, in_maps, core_ids=list(range(8)))
    return assemble_output(res.results, orders).astype(np.float32)
